# revision 6
# baseline (speedup 1.0000x reference)
# Trainium2 Bass kernel for nn_CNF: conditional CNF log-density via 8-step RK4
# with exact-trace divergence.
#
# Math: the reference's vjp-based trace of the Jacobian has a closed form for
# this 3-layer ConcatSquash MLP:
#   h1 = tanh(g0*(W1_0@[y;c]) + c0),  h2 = tanh(g1*(W1_1@h1) + c1),
#   f  = g2*(W1_2@h2) + c2,           (g*, c* depend only on t)
#   tr(df/dy) = (1-h1^2)^T Qhat (1-h2^2),
#   Qhat[k,j] = g0[k] * (W0y diag(g2) W1_2)[k,j] * W1_1[j,k] * g1[j]
# All t-dependent matrices are precomputed on the host (they are shared by
# every sample); the device only runs the per-sample batched work.
#
# The RK4 intermediate state never materializes: with U0 = W1_0@[y;c],
#   U0_{e+1} = U0_base + G_e @ h2_e + d_e,
#   G_e = alpha_e * W0y diag(g2_e) W1_2,   d_e = alpha_e * W0y @ c2_e
# so each eval is: matmul-group -> tanh -> matmul -> tanh (+ trace side ops).
#
# Sharding: pure data parallelism, batch 2048 -> 8 cores x 256 samples.

import json
import numpy as np

import concourse.bass as bass
import concourse.mybir as mybir
import concourse.tile as tile
import concourse.bass2jax as bass2jax
from concourse.vector_clock import ScopedClock
from concourse.bass_utils import run_bass_kernel_spmd

F32 = mybir.dt.float32
F32R = mybir.dt.float32r
AF = mybir.ActivationFunctionType
ALU = mybir.AluOpType

L = 16
C = 16
WID = 128
B = 2048
NCORES = 8
BC = B // NCORES          # 256 samples per core
NSTEPS = 8
NEVALS = 4 * NSTEPS       # 32
DT = -1.0 / NSTEPS
LOG2PI = float(np.log(2.0 * np.pi))

# ---------------------------------------------------------------------------
# Workarounds: walrus in this container encodes at most ONE sync-wait command
# per instruction. (1) split the Tile tail-drain's waits over multiple drains;
# (2) split any instruction's excess waits onto preceding EventSemaphore
# instructions at BIR-JSON level inside the compile hook.
# ---------------------------------------------------------------------------
_MAX_WAITS = 1


def _patched_drain_and_barrier(self, tick_clock, wait_clock):
    nc = self.nc
    drain_inst = nc.sync.drain()
    wait_clock.add_sem_waits(
        drain_inst.ins, ScopedClock({None: tick_clock.global_clock})
    )
    si = drain_inst.ins.sync_info
    if si is not None:
        waits = list(si.on_wait)
        if len(waits) > _MAX_WAITS:
            drain_inst.ins.sync_info = mybir.SyncInfo(
                on_wait=list(waits[:_MAX_WAITS]), on_update=list(si.on_update)
            )
            for i in range(_MAX_WAITS, len(waits), _MAX_WAITS):
                extra = nc.sync.drain()
                extra.ins.sync_info = mybir.SyncInfo(
                    on_wait=list(waits[i:i + _MAX_WAITS]), on_update=[]
                )
    nc.all_engine_barrier()
    assert self.sems is not None
    popped = nc._tile_sem_poison_stack.pop()
    assert popped is self._sem_poison
    nc.clear_and_free_semaphores(list(self.sems.allocated().values()))
    nc.all_engine_barrier()


tile.TileContext._drain_and_barrier = _patched_drain_and_barrier


def _split_excess_waits(bir_bytes):
    m = json.loads(bir_bytes)
    changed = False
    ctr = 0
    for fn in m.get("functions", []):
        for blk in fn.get("blocks", []):
            insts = blk.get("instructions", [])
            out = []
            for inst in insts:
                si = inst.get("sync_info")
                if si:
                    waits = si.get("on_wait") or []
                    if len(waits) > _MAX_WAITS:
                        for wt in waits[:-_MAX_WAITS]:
                            ctr += 1
                            out.append({
                                "name": f"xw-{ctr}",
                                "opcode": "EventSemaphore",
                                "engine": inst["engine"],
                                "ins": [], "outs": [],
                                "sync_info": {"on_wait": [wt], "on_update": []},
                            })
                        si["on_wait"] = waits[-_MAX_WAITS:]
                        changed = True
                out.append(inst)
            if changed:
                blk["instructions"] = out
    if not changed:
        return bir_bytes
    return json.dumps(m).encode()


if not getattr(bass2jax, "_ant_wait_split_patched", False):
    _orig_compile_bir_kernel = bass2jax.compile_bir_kernel

    def _patched_compile_bir_kernel(bir_json, tmpdir, neff_name="file.neff"):
        return _orig_compile_bir_kernel(
            _split_excess_waits(bir_json), tmpdir, neff_name
        )

    bass2jax.compile_bir_kernel = _patched_compile_bir_kernel
    bass2jax._ant_wait_split_patched = True


# ---------------------------------------------------------------------------
# Host-side precompute of all t-dependent constants.
# ---------------------------------------------------------------------------
def _sigmoid(x):
    return 1.0 / (1.0 + np.exp(-x))


def _precompute(ws):
    W1_0 = ws["W1_0"].astype(np.float64)
    b1_0 = ws["b1_0"].astype(np.float64)
    W2_0 = ws["W2_0"].astype(np.float64)[:, 0]
    b2_0 = ws["b2_0"].astype(np.float64)
    W3_0 = ws["W3_0"].astype(np.float64)[:, 0]
    W1_1 = ws["W1_1"].astype(np.float64)
    b1_1 = ws["b1_1"].astype(np.float64)
    W2_1 = ws["W2_1"].astype(np.float64)[:, 0]
    b2_1 = ws["b2_1"].astype(np.float64)
    W3_1 = ws["W3_1"].astype(np.float64)[:, 0]
    W1_2 = ws["W1_2"].astype(np.float64)
    b1_2 = ws["b1_2"].astype(np.float64)
    W2_2 = ws["W2_2"].astype(np.float64)[:, 0]
    b2_2 = ws["b2_2"].astype(np.float64)
    W3_2 = ws["W3_2"].astype(np.float64)[:, 0]

    W0y = W1_0[:, :L]                       # [128, 16]
    alphas = [DT / 2.0, DT / 2.0, DT]       # alpha_e, e = 0,1,2
    wrk4 = [DT / 6.0, DT / 3.0, DT / 3.0, DT / 6.0]
    toffs = [0.0, DT / 2.0, DT / 2.0, DT]

    wbase = np.zeros((33, NEVALS * WID), np.float64)   # lhsT per eval
    wg = np.zeros((WID, NSTEPS * 3 * WID), np.float64)
    wtr = np.zeros((WID, NEVALS * WID), np.float64)
    wl2 = np.zeros((WID, NEVALS * L), np.float64)
    wdc = np.zeros((WID, NEVALS), np.float64)
    rs_all = np.zeros((WID, NEVALS), np.float64)
    actc = np.zeros((WID, 4 * NEVALS), np.float64)
    wc2w = np.zeros((1, NSTEPS * L), np.float64)
    s0_total = 0.0

    W10T = W1_0.T                            # [32, 128]
    prev = None                              # (alpha, g2, c2) of previous eval
    for n in range(NSTEPS):
        t0 = 1.0 + n * DT
        c2wn = np.zeros(L)
        for e in range(4):
            i = 4 * n + e
            t = t0 + toffs[e]
            g0 = _sigmoid(W2_0 * t + b2_0)
            c0 = b1_0 * g0 + W3_0 * t
            g1 = _sigmoid(W2_1 * t + b2_1)
            c1 = b1_1 * g1 + W3_1 * t
            g2 = _sigmoid(W2_2 * t + b2_2)
            c2 = b1_2 * g2 + W3_2 * t

            # base lhsT [33, 128]: rows 0-31 = W1_0^T, row 32 = d_{prev}
            wbase[:32, i * WID:(i + 1) * WID] = W10T
            if e > 0:
                al, g2p, c2p = prev
                wbase[32, i * WID:(i + 1) * WID] = al * (W0y @ c2p)
                G = al * (W0y @ (g2p[:, None] * W1_2))       # [128(m),128(j)]
                gi = (n * 3 + (e - 1)) * WID
                wg[:, gi:gi + WID] = G.T                     # lhsT [j, m]

            Qraw = (W0y @ (g2[:, None] * W1_2)) * W1_1.T     # [k, j]
            Qhat = wrk4[e] * (g0[:, None] * Qraw * g1[None, :])
            rs_all[:, i] = Qhat.sum(axis=1)
            wdc[:, i] = -Qhat.sum(axis=0)
            s0_total += Qhat.sum()
            wtr[:, i * WID:(i + 1) * WID] = Qhat.T           # lhsT [j, k]

            wl2[:, i * L:(i + 1) * L] = wrk4[e] * (g2[None, :] * W1_2.T)
            c2wn += wrk4[e] * c2

            actc[:, 4 * i + 0] = g0
            actc[:, 4 * i + 1] = c0
            actc[:, 4 * i + 2] = g1
            actc[:, 4 * i + 3] = c1
            prev = (alphas[e], g2, c2) if e < 3 else None
        wc2w[0, n * L:(n + 1) * L] = c2wn

    f32 = np.float32
    consts = {
        "wbase": wbase.astype(f32), "wg": wg.astype(f32),
        "wtr": wtr.astype(f32), "wl2": wl2.astype(f32),
        "wdc": wdc.astype(f32), "rs": rs_all.astype(f32),
        "actc": actc.astype(f32), "wc2w": wc2w.astype(f32),
        "onesw": np.ones((WID, 1), f32),
        "negh": np.full((L, 1), -0.5, f32),
    }
    const_add = np.float64(s0_total) - (L / 2.0) * LOG2PI
    return consts, np.float32(const_add)


# ---------------------------------------------------------------------------
# Device program (built once per process).
# ---------------------------------------------------------------------------
_prog_cache = {}


def _build_program():
    if "nc" in _prog_cache:
        return _prog_cache["nc"]

    nc = bass.Bass()
    d_xb = nc.dram_tensor("xb0", [33, BC], F32R, kind="ExternalInput")
    d_wbase = nc.dram_tensor("wbase", [33, NEVALS * WID], F32R, kind="ExternalInput")
    d_wg = nc.dram_tensor("wg", [WID, NSTEPS * 3 * WID], F32R, kind="ExternalInput")
    d_wtr = nc.dram_tensor("wtr", [WID, NEVALS * WID], F32R, kind="ExternalInput")
    d_wl2 = nc.dram_tensor("wl2", [WID, NEVALS * L], F32R, kind="ExternalInput")
    d_wdc = nc.dram_tensor("wdc", [WID, NEVALS], F32R, kind="ExternalInput")
    d_rs = nc.dram_tensor("rs", [WID, NEVALS], F32, kind="ExternalInput")
    d_actc = nc.dram_tensor("actc", [WID, 4 * NEVALS], F32, kind="ExternalInput")
    d_wc2w = nc.dram_tensor("wc2w", [1, NSTEPS * L], F32R, kind="ExternalInput")
    d_onesw = nc.dram_tensor("onesw", [WID, 1], F32R, kind="ExternalInput")
    d_negh = nc.dram_tensor("negh", [L, 1], F32R, kind="ExternalInput")
    d_wl1 = nc.dram_tensor("wl1", [WID, WID], F32R, kind="ExternalInput")
    d_onesrow = nc.dram_tensor("onesrow", [1, BC], F32R, kind="ExternalInput")
    d_out = nc.dram_tensor("out", [1, BC], F32, kind="ExternalOutput")

    with tile.TileContext(nc) as tc:
        with tc.tile_pool(name="consts", bufs=1) as cp, \
             tc.tile_pool(name="work", bufs=2) as wp, \
             tc.tile_pool(name="u0p", bufs=2, space="PSUM") as u0p, \
             tc.tile_pool(name="u1p", bufs=2, space="PSUM") as u1p, \
             tc.tile_pool(name="vp", bufs=2, space="PSUM") as vp, \
             tc.tile_pool(name="yp", bufs=1, space="PSUM") as yp, \
             tc.tile_pool(name="dp", bufs=1, space="PSUM") as dp:

            xb = cp.tile([33, BC], F32R)
            nc.sync.dma_start(xb[:], d_xb[:])
            wl1t = cp.tile([WID, WID], F32R)
            nc.sync.dma_start(wl1t[:], d_wl1[:])
            actc = cp.tile([WID, 4 * NEVALS], F32)
            nc.sync.dma_start(actc[:], d_actc[:])
            rs = cp.tile([WID, NEVALS], F32)
            nc.sync.dma_start(rs[:], d_rs[:])
            wdc = cp.tile([WID, NEVALS], F32R)
            nc.sync.dma_start(wdc[:], d_wdc[:])
            onesw = cp.tile([WID, 1], F32R)
            nc.sync.dma_start(onesw[:], d_onesw[:])
            negh = cp.tile([L, 1], F32R)
            nc.sync.dma_start(negh[:], d_negh[:])
            wc2w = cp.tile([1, NSTEPS * L], F32R)
            nc.sync.dma_start(wc2w[:], d_wc2w[:])
            onesrow = cp.tile([1, BC], F32R)
            nc.sync.dma_start(onesrow[:], d_onesrow[:])
            wl2 = cp.tile([WID, NEVALS * L], F32R)
            nc.sync.dma_start(wl2[:], d_wl2[:])

            # big consts: load in chunks so early evals unblock early
            wbase = cp.tile([33, NEVALS * WID], F32R)
            wtr = cp.tile([WID, NEVALS * WID], F32R)
            wg = cp.tile([WID, NSTEPS * 3 * WID], F32R)
            NCH = 8
            cw = NEVALS * WID // NCH
            for ch in range(NCH):
                sl = slice(ch * cw, (ch + 1) * cw)
                nc.sync.dma_start(wbase[:, sl], d_wbase[:, sl])
                nc.sync.dma_start(wtr[:, sl], d_wtr[:, sl])
            gw = NSTEPS * 3 * WID // NCH
            for ch in range(NCH):
                sl = slice(ch * gw, (ch + 1) * gw)
                nc.sync.dma_start(wg[:, sl], d_wg[:, sl])

            ps_dlp = dp.tile([1, BC], F32)
            h2_prev = None
            for n in range(NSTEPS):
                ps_yacc = yp.tile([L, BC], F32)
                for e in range(4):
                    i = 4 * n + e
                    # --- layer0 pre-activation group ---
                    ps_u0 = u0p.tile([WID, BC], F32, tag="u0")
                    nc.tensor.matmul(
                        ps_u0[:], wbase[:, i * WID:(i + 1) * WID], xb[:],
                        start=True, stop=(e == 0))
                    if e > 0:
                        gi = (n * 3 + (e - 1)) * WID
                        nc.tensor.matmul(
                            ps_u0[:], wg[:, gi:gi + WID], h2_prev[:],
                            start=False, stop=True)
                    # --- h1 = tanh(g0*u0 + c0) ---
                    h1 = wp.tile([WID, BC], F32R, tag="h1")
                    nc.scalar.activation(
                        h1[:], ps_u0[:], AF.Tanh,
                        bias=actc[:, 4 * i + 1:4 * i + 2],
                        scale=actc[:, 4 * i + 0:4 * i + 1])
                    # --- layer1 ---
                    ps_u1 = u1p.tile([WID, BC], F32, tag="u1")
                    nc.tensor.matmul(ps_u1[:], wl1t[:], h1[:],
                                     start=True, stop=True)
                    # --- sq1 = h1^2 (ScalarE) ---
                    sq1 = wp.tile([WID, BC], F32R, tag="sq1")
                    nc.scalar.activation(sq1[:], h1[:].bitcast(F32), AF.Square)
                    # --- h2 = tanh(g1*u1 + c1) ---
                    h2 = wp.tile([WID, BC], F32R, tag="h2")
                    nc.scalar.activation(
                        h2[:], ps_u1[:], AF.Tanh,
                        bias=actc[:, 4 * i + 3:4 * i + 4],
                        scale=actc[:, 4 * i + 2:4 * i + 3])
                    # --- sq2 = h2^2 (VectorE) ---
                    sq2 = wp.tile([WID, BC], F32R, tag="sq2")
                    nc.vector.tensor_mul(sq2[:], h2[:].bitcast(F32),
                                         h2[:].bitcast(F32))
                    # --- trace side ---
                    ps_v = vp.tile([WID, BC], F32, tag="v")
                    nc.tensor.matmul(ps_v[:], wtr[:, i * WID:(i + 1) * WID],
                                     sq2[:], start=True, stop=True)
                    nc.tensor.matmul(ps_dlp[:], wdc[:, i:i + 1], sq2[:],
                                     start=(i == 0), stop=False)
                    w = wp.tile([WID, BC], F32R, tag="w")
                    nc.vector.scalar_tensor_tensor(
                        w[:], ps_v[:], rs[:, i:i + 1], sq1[:].bitcast(F32),
                        op0=ALU.subtract, op1=ALU.mult)
                    nc.tensor.matmul(ps_dlp[:], onesw[:], w[:],
                                     start=False, stop=False)
                    # --- y accumulation ---
                    nc.tensor.matmul(ps_yacc[:], wl2[:, i * L:(i + 1) * L],
                                     h2[:], start=(e == 0), stop=(e == 3))
                    if e == 0:
                        nc.tensor.matmul(ps_yacc[:],
                                         wc2w[:, n * L:(n + 1) * L],
                                         onesrow[:], start=False,
                                         stop=False)
                    h2_prev = h2
                # --- step epilogue: y += yacc (in fp32) ---
                nc.vector.tensor_add(xb[0:L, :], ps_yacc[:],
                                     xb[0:L, :].bitcast(F32))
            # --- finish: base logp of y_final ---
            sqy = wp.tile([L, BC], F32R, tag="sqy")
            nc.vector.tensor_mul(sqy[:], xb[0:L, :].bitcast(F32),
                                 xb[0:L, :].bitcast(F32))
            nc.tensor.matmul(ps_dlp[:], negh[:], sqy[:],
                             start=False, stop=True)
            outs = wp.tile([1, BC], F32, tag="outs")
            nc.scalar.copy(outs[:], ps_dlp[:])
            nc.sync.dma_start(d_out[:], outs[:])

    _prog_cache["nc"] = nc
    return nc


# ---------------------------------------------------------------------------
# Public entry point.
# ---------------------------------------------------------------------------
def _run(inputs, **spmd_kwargs):
    z = np.ascontiguousarray(inputs["z"], dtype=np.float32)
    cond = np.ascontiguousarray(inputs["cond"], dtype=np.float32)
    ws = {k: np.asarray(v) for k, v in inputs.items() if k not in ("z", "cond")}

    consts, const_add = _precompute(ws)
    # wl1 lhsT = W1_1^T
    wl1 = np.ascontiguousarray(ws["W1_1"].astype(np.float32).T)

    nc = _build_program()

    in_maps = []
    for cix in range(NCORES):
        sl = slice(cix * BC, (cix + 1) * BC)
        xb0 = np.concatenate(
            [z[sl].T, cond[sl].T, np.ones((1, BC), np.float32)], axis=0)
        im = {"xb0": np.ascontiguousarray(xb0), "wl1": wl1,
              "onesrow": np.ones((1, BC), np.float32)}
        im.update(consts)
        in_maps.append(im)

    res = run_bass_kernel_spmd(nc, in_maps, core_ids=list(range(NCORES)),
                               **spmd_kwargs)
    out = np.concatenate(
        [res.results[cix]["out"][0] for cix in range(NCORES)])
    return (out + const_add).astype(np.float32), res


def kernel(**inputs):
    out, _ = _run(inputs)
    return out


if __name__ == "__main__":
    rng = np.random.default_rng(0)
    fake = {}
    sizes = [(WID, L + C), (WID, WID), (L, WID)]
    for idx, (o, inp) in enumerate(sizes):
        fake[f"W1_{idx}"] = rng.standard_normal((o, inp)).astype(np.float32) * 0.1
        fake[f"b1_{idx}"] = rng.standard_normal(o).astype(np.float32) * 0.1
        fake[f"W2_{idx}"] = rng.standard_normal((o, 1)).astype(np.float32) * 0.1
        fake[f"b2_{idx}"] = rng.standard_normal(o).astype(np.float32) * 0.1
        fake[f"W3_{idx}"] = rng.standard_normal((o, 1)).astype(np.float32) * 0.1
    fake["z"] = rng.standard_normal((B, L)).astype(np.float32)
    fake["cond"] = rng.standard_normal((B, C)).astype(np.float32)
    print(kernel(**fake)[:8])


# revision 34
# speedup vs baseline: 8731.0435x; 8731.0435x over previous
# Trainium2 Bass kernel for nn_CNF: conditional CNF log-density via 8-step RK4
# with exact-trace divergence.
#
# Math: the reference's vjp-based trace of the Jacobian has a closed form for
# this 3-layer ConcatSquash MLP:
#   h1 = tanh(g0*(W1_0@[y;c]) + c0),  h2 = tanh(g1*(W1_1@h1) + c1),
#   f  = g2*(W1_2@h2) + c2,           (g*, c* depend only on t)
#   tr(df/dy) = (1-h1^2)^T Qhat (1-h2^2),
#   Qhat[k,j] = g0[k] * (W0y diag(g2) W1_2)[k,j] * W1_1[j,k] * g1[j]
# All t-dependent matrices are precomputed on the host (they are shared by
# every sample); the device only runs the per-sample batched work.
#
# The RK4 intermediate state never materializes: with U0 = W1_0@[y;c],
#   U0_{e+1} = U0_base + G_e @ h2_e + d_e,
#   G_e = alpha_e * W0y diag(g2_e) W1_2,   d_e = alpha_e * W0y @ c2_e
# so each eval is: matmul-group -> tanh -> matmul -> tanh (+ trace side ops).
#
# Sharding: pure data parallelism, batch 2048 -> 8 cores x 256 samples.

import json
import numpy as np

import concourse.bass as bass
import concourse.mybir as mybir
import concourse.tile as tile
import concourse.bass2jax as bass2jax
from concourse.vector_clock import ScopedClock
from concourse.bass_utils import run_bass_kernel_spmd

F32 = mybir.dt.float32
F32R = mybir.dt.float32r
AF = mybir.ActivationFunctionType
ALU = mybir.AluOpType

L = 16
C = 16
WID = 128
B = 2048
NCORES = 8
BC = B // NCORES          # 256 samples per core
NSTEPS = 8
NEVALS = 4 * NSTEPS       # 32
DT = -1.0 / NSTEPS
LOG2PI = float(np.log(2.0 * np.pi))

# ---------------------------------------------------------------------------
# Workarounds: walrus in this container encodes at most ONE sync-wait command
# per instruction. (1) split the Tile tail-drain's waits over multiple drains;
# (2) split any instruction's excess waits onto preceding EventSemaphore
# instructions at BIR-JSON level inside the compile hook.
# ---------------------------------------------------------------------------
_MAX_WAITS = 1


def _patched_drain_and_barrier(self, tick_clock, wait_clock):
    nc = self.nc
    drain_inst = nc.sync.drain()
    wait_clock.add_sem_waits(
        drain_inst.ins, ScopedClock({None: tick_clock.global_clock})
    )
    si = drain_inst.ins.sync_info
    if si is not None:
        waits = list(si.on_wait)
        if len(waits) > _MAX_WAITS:
            drain_inst.ins.sync_info = mybir.SyncInfo(
                on_wait=list(waits[:_MAX_WAITS]), on_update=list(si.on_update)
            )
            for i in range(_MAX_WAITS, len(waits), _MAX_WAITS):
                extra = nc.sync.drain()
                extra.ins.sync_info = mybir.SyncInfo(
                    on_wait=list(waits[i:i + _MAX_WAITS]), on_update=[]
                )
    nc.all_engine_barrier()
    assert self.sems is not None
    popped = nc._tile_sem_poison_stack.pop()
    assert popped is self._sem_poison
    nc.clear_and_free_semaphores(list(self.sems.allocated().values()))
    nc.all_engine_barrier()


tile.TileContext._drain_and_barrier = _patched_drain_and_barrier


def _split_excess_waits(bir_bytes):
    m = json.loads(bir_bytes)
    changed = False
    ctr = 0
    for fn in m.get("functions", []):
        for blk in fn.get("blocks", []):
            insts = blk.get("instructions", [])
            out = []
            for inst in insts:
                si = inst.get("sync_info")
                if si:
                    waits = si.get("on_wait") or []
                    if len(waits) > _MAX_WAITS:
                        for wt in waits[:-_MAX_WAITS]:
                            ctr += 1
                            out.append({
                                "name": f"xw-{ctr}",
                                "opcode": "EventSemaphore",
                                "engine": inst["engine"],
                                "ins": [], "outs": [],
                                "sync_info": {"on_wait": [wt], "on_update": []},
                            })
                        si["on_wait"] = waits[-_MAX_WAITS:]
                        changed = True
                out.append(inst)
            if changed:
                blk["instructions"] = out
    if not changed:
        return bir_bytes
    return json.dumps(m).encode()


if not getattr(bass2jax, "_ant_wait_split_patched", False):
    _orig_compile_bir_kernel = bass2jax.compile_bir_kernel

    def _patched_compile_bir_kernel(bir_json, tmpdir, neff_name="file.neff"):
        return _orig_compile_bir_kernel(
            _split_excess_waits(bir_json), tmpdir, neff_name
        )

    bass2jax.compile_bir_kernel = _patched_compile_bir_kernel
    bass2jax._ant_wait_split_patched = True


# ---------------------------------------------------------------------------
# Host-side precompute of all t-dependent constants.
# ---------------------------------------------------------------------------
def _sigmoid(x):
    return 1.0 / (1.0 + np.exp(-x))


def _precompute(ws):
    W1_0 = ws["W1_0"].astype(np.float64)
    b1_0 = ws["b1_0"].astype(np.float64)
    W2_0 = ws["W2_0"].astype(np.float64)[:, 0]
    b2_0 = ws["b2_0"].astype(np.float64)
    W3_0 = ws["W3_0"].astype(np.float64)[:, 0]
    W1_1 = ws["W1_1"].astype(np.float64)
    b1_1 = ws["b1_1"].astype(np.float64)
    W2_1 = ws["W2_1"].astype(np.float64)[:, 0]
    b2_1 = ws["b2_1"].astype(np.float64)
    W3_1 = ws["W3_1"].astype(np.float64)[:, 0]
    W1_2 = ws["W1_2"].astype(np.float64)
    b1_2 = ws["b1_2"].astype(np.float64)
    W2_2 = ws["W2_2"].astype(np.float64)[:, 0]
    b2_2 = ws["b2_2"].astype(np.float64)
    W3_2 = ws["W3_2"].astype(np.float64)[:, 0]

    W0y = W1_0[:, :L]                       # [128, 16]
    alphas = [DT / 2.0, DT / 2.0, DT]       # alpha_e, e = 0,1,2
    wrk4 = [DT / 6.0, DT / 3.0, DT / 3.0, DT / 6.0]
    toffs = [0.0, DT / 2.0, DT / 2.0, DT]

    wbase = np.zeros((33, NEVALS * WID), np.float64)   # lhsT per eval
    wg = np.zeros((WID, NSTEPS * 3 * WID), np.float64)
    wtr = np.zeros((WID, NEVALS * WID), np.float64)
    wl2 = np.zeros((WID, NEVALS * L), np.float64)
    wdc = np.zeros((WID, NEVALS), np.float64)
    rs_all = np.zeros((WID, NEVALS), np.float64)
    actc = np.zeros((WID, 4 * NEVALS), np.float64)
    wc2w = np.zeros((1, NSTEPS * L), np.float64)
    s0_total = 0.0

    W10T = W1_0.T                            # [32, 128]
    prev = None                              # (alpha, g2, c2) of previous eval
    for n in range(NSTEPS):
        t0 = 1.0 + n * DT
        c2wn = np.zeros(L)
        for e in range(4):
            i = 4 * n + e
            t = t0 + toffs[e]
            g0 = _sigmoid(W2_0 * t + b2_0)
            c0 = b1_0 * g0 + W3_0 * t
            g1 = _sigmoid(W2_1 * t + b2_1)
            c1 = b1_1 * g1 + W3_1 * t
            g2 = _sigmoid(W2_2 * t + b2_2)
            c2 = b1_2 * g2 + W3_2 * t

            # base lhsT [33, 128]: rows 0-31 = W1_0^T, row 32 = d_{prev}
            wbase[:32, i * WID:(i + 1) * WID] = W10T
            if e > 0:
                al, g2p, c2p = prev
                wbase[32, i * WID:(i + 1) * WID] = al * (W0y @ c2p)
                G = al * (W0y @ (g2p[:, None] * W1_2))       # [128(m),128(j)]
                gi = (n * 3 + (e - 1)) * WID
                wg[:, gi:gi + WID] = G.T                     # lhsT [j, m]

            Qraw = (W0y @ (g2[:, None] * W1_2)) * W1_1.T     # [k, j]
            Qhat = wrk4[e] * (g0[:, None] * Qraw * g1[None, :])
            rs_all[:, i] = Qhat.sum(axis=1)
            wdc[:, i] = -Qhat.sum(axis=0)
            s0_total += Qhat.sum()
            wtr[:, i * WID:(i + 1) * WID] = Qhat.T           # lhsT [j, k]

            wl2[:, i * L:(i + 1) * L] = wrk4[e] * (g2[None, :] * W1_2.T)
            c2wn += wrk4[e] * c2

            actc[:, 4 * i + 0] = g0
            actc[:, 4 * i + 1] = c0
            actc[:, 4 * i + 2] = g1
            actc[:, 4 * i + 3] = c1
            prev = (alphas[e], g2, c2) if e < 3 else None
        wc2w[0, n * L:(n + 1) * L] = c2wn

    f32 = np.float32
    consts = {
        "wbase": wbase.astype(f32), "wg": wg.astype(f32),
        "wtr": wtr.astype(f32), "wl2": wl2.astype(f32),
        "wdc": wdc.astype(f32), "rs": rs_all.astype(f32),
        "actc": actc.astype(f32), "wc2w": wc2w.astype(f32),
        "onesw": np.ones((WID, 1), f32),
        "negh": np.full((L, 1), -0.5, f32),
    }
    const_add = np.float64(s0_total) - (L / 2.0) * LOG2PI
    return consts, np.float32(const_add)


# ---------------------------------------------------------------------------
# Device program (built once per process).
# ---------------------------------------------------------------------------
_prog_cache = {}


def _build_program():
    if "nc" in _prog_cache:
        return _prog_cache["nc"]

    nc = bass.Bass()
    d_xb = nc.dram_tensor("xb0", [33, BC], F32R, kind="ExternalInput")
    d_wbase = nc.dram_tensor("wbase", [33, NEVALS * WID], F32R, kind="ExternalInput")
    d_wg = nc.dram_tensor("wg", [WID, NSTEPS * 3 * WID], F32R, kind="ExternalInput")
    d_wtr = nc.dram_tensor("wtr", [WID, NEVALS * WID], F32R, kind="ExternalInput")
    d_wl2 = nc.dram_tensor("wl2", [WID, NEVALS * L], F32R, kind="ExternalInput")
    d_wdc = nc.dram_tensor("wdc", [WID, NEVALS], F32R, kind="ExternalInput")
    d_rs = nc.dram_tensor("rs", [WID, NEVALS], F32, kind="ExternalInput")
    d_actc = nc.dram_tensor("actc", [WID, 4 * NEVALS], F32, kind="ExternalInput")
    d_wc2w = nc.dram_tensor("wc2w", [1, NSTEPS * L], F32R, kind="ExternalInput")
    d_onesw = nc.dram_tensor("onesw", [WID, 1], F32R, kind="ExternalInput")
    d_negh = nc.dram_tensor("negh", [L, 1], F32R, kind="ExternalInput")
    d_wl1 = nc.dram_tensor("wl1", [WID, WID], F32R, kind="ExternalInput")
    d_onesrow = nc.dram_tensor("onesrow", [1, BC], F32R, kind="ExternalInput")
    d_out = nc.dram_tensor("out", [1, BC], F32, kind="ExternalOutput")

    with tile.TileContext(nc) as tc:
        with tc.tile_pool(name="consts", bufs=1) as cp, \
             tc.tile_pool(name="work", bufs=2) as wp, \
             tc.tile_pool(name="u0p", bufs=2, space="PSUM") as u0p, \
             tc.tile_pool(name="u1p", bufs=2, space="PSUM") as u1p, \
             tc.tile_pool(name="vp", bufs=2, space="PSUM") as vp, \
             tc.tile_pool(name="yp", bufs=1, space="PSUM") as yp, \
             tc.tile_pool(name="dp", bufs=1, space="PSUM") as dp:

            xb = cp.tile([33, BC], F32R)
            nc.sync.dma_start(xb[:], d_xb[:])
            wl1t = cp.tile([WID, WID], F32R)
            nc.sync.dma_start(wl1t[:], d_wl1[:])
            actc = cp.tile([WID, 4 * NEVALS], F32)
            nc.sync.dma_start(actc[:], d_actc[:])
            rs = cp.tile([WID, NEVALS], F32)
            nc.sync.dma_start(rs[:], d_rs[:])
            wdc = cp.tile([WID, NEVALS], F32R)
            nc.sync.dma_start(wdc[:], d_wdc[:])
            onesw = cp.tile([WID, 1], F32R)
            nc.sync.dma_start(onesw[:], d_onesw[:])
            negh = cp.tile([L, 1], F32R)
            nc.sync.dma_start(negh[:], d_negh[:])
            wc2w = cp.tile([1, NSTEPS * L], F32R)
            nc.sync.dma_start(wc2w[:], d_wc2w[:])
            onesrow = cp.tile([1, BC], F32R)
            nc.sync.dma_start(onesrow[:], d_onesrow[:])
            wl2 = cp.tile([WID, NEVALS * L], F32R)
            nc.sync.dma_start(wl2[:], d_wl2[:])

            # big consts: load in chunks so early evals unblock early
            wbase = cp.tile([33, NEVALS * WID], F32R)
            wtr = cp.tile([WID, NEVALS * WID], F32R)
            wg = cp.tile([WID, NSTEPS * 3 * WID], F32R)
            NCH = 8
            cw = NEVALS * WID // NCH
            for ch in range(NCH):
                sl = slice(ch * cw, (ch + 1) * cw)
                nc.sync.dma_start(wbase[:, sl], d_wbase[:, sl])
                nc.sync.dma_start(wtr[:, sl], d_wtr[:, sl])
            gw = NSTEPS * 3 * WID // NCH
            for ch in range(NCH):
                sl = slice(ch * gw, (ch + 1) * gw)
                nc.sync.dma_start(wg[:, sl], d_wg[:, sl])

            ps_dlp = dp.tile([1, BC], F32)
            state = {"h2_prev": None}

            def emit_solve(first):
                for n in range(NSTEPS):
                    ps_yacc = yp.tile([L, BC], F32, tag="yacc")
                    for e in range(4):
                        i = 4 * n + e
                        # --- layer0 pre-activation group ---
                        ps_u0 = u0p.tile([WID, BC], F32, tag="u0")
                        nc.tensor.matmul(
                            ps_u0[:], wbase[:, i * WID:(i + 1) * WID], xb[:],
                            start=True, stop=(e == 0))
                        if e > 0:
                            gi = (n * 3 + (e - 1)) * WID
                            nc.tensor.matmul(
                                ps_u0[:], wg[:, gi:gi + WID],
                                state["h2_prev"][:], start=False, stop=True)
                        # --- h1 = tanh(g0*u0 + c0) ---
                        h1 = wp.tile([WID, BC], F32R, tag="h1")
                        nc.scalar.activation(
                            h1[:], ps_u0[:], AF.Tanh,
                            bias=actc[:, 4 * i + 1:4 * i + 2],
                            scale=actc[:, 4 * i + 0:4 * i + 1])
                        # --- layer1 ---
                        ps_u1 = u1p.tile([WID, BC], F32, tag="u1")
                        nc.tensor.matmul(ps_u1[:], wl1t[:], h1[:],
                                         start=True, stop=True)
                        # --- sq1 = h1^2 (ScalarE) ---
                        sq1 = wp.tile([WID, BC], F32R, tag="sq1")
                        nc.scalar.activation(sq1[:], h1[:].bitcast(F32),
                                             AF.Square)
                        # --- h2 = tanh(g1*u1 + c1) ---
                        h2 = wp.tile([WID, BC], F32R, tag="h2")
                        nc.scalar.activation(
                            h2[:], ps_u1[:], AF.Tanh,
                            bias=actc[:, 4 * i + 3:4 * i + 4],
                            scale=actc[:, 4 * i + 2:4 * i + 3])
                        # --- sq2 = h2^2 (VectorE) ---
                        sq2 = wp.tile([WID, BC], F32R, tag="sq2")
                        nc.vector.tensor_mul(sq2[:], h2[:].bitcast(F32),
                                             h2[:].bitcast(F32))
                        # --- trace side ---
                        ps_v = vp.tile([WID, BC], F32, tag="v")
                        nc.tensor.matmul(ps_v[:],
                                         wtr[:, i * WID:(i + 1) * WID],
                                         sq2[:], start=True, stop=True)
                        nc.tensor.matmul(ps_dlp[:], wdc[:, i:i + 1], sq2[:],
                                         start=(first and i == 0), stop=False)
                        w = wp.tile([WID, BC], F32R, tag="w")
                        nc.vector.scalar_tensor_tensor(
                            w[:], ps_v[:], rs[:, i:i + 1], sq1[:].bitcast(F32),
                            op0=ALU.subtract, op1=ALU.mult)
                        nc.tensor.matmul(ps_dlp[:], onesw[:], w[:],
                                         start=False, stop=False)
                        # --- y accumulation ---
                        nc.tensor.matmul(ps_yacc[:],
                                         wl2[:, i * L:(i + 1) * L],
                                         h2[:], start=(e == 0), stop=(e == 3))
                        if e == 0:
                            nc.tensor.matmul(ps_yacc[:],
                                             wc2w[:, n * L:(n + 1) * L],
                                             onesrow[:], start=False,
                                             stop=False)
                        state["h2_prev"] = h2
                    # --- step epilogue: y += yacc (in fp32) ---
                    nc.vector.tensor_add(xb[0:L, :], ps_yacc[:],
                                         xb[0:L, :].bitcast(F32))

            if loop_iters:
                with tc.For_i(0, loop_iters, 1):
                    emit_solve(False)
            else:
                for rep in range(repeats):
                    emit_solve(rep == 0)
            # --- finish: base logp of y_final ---
            sqy = wp.tile([L, BC], F32R, tag="sqy")
            nc.vector.tensor_mul(sqy[:], xb[0:L, :].bitcast(F32),
                                 xb[0:L, :].bitcast(F32))
            nc.tensor.matmul(ps_dlp[:], negh[:], sqy[:],
                             start=False, stop=True)
            outs = wp.tile([1, BC], F32, tag="outs")
            nc.scalar.copy(outs[:], ps_dlp[:])
            nc.sync.dma_start(d_out[:], outs[:])

    _prog_cache["nc"] = nc
    return nc


# ---------------------------------------------------------------------------
# Public entry point.
# ---------------------------------------------------------------------------
def _run(inputs, **spmd_kwargs):
    z = np.ascontiguousarray(inputs["z"], dtype=np.float32)
    cond = np.ascontiguousarray(inputs["cond"], dtype=np.float32)
    ws = {k: np.asarray(v) for k, v in inputs.items() if k not in ("z", "cond")}

    consts, const_add = _precompute(ws)
    # wl1 lhsT = W1_1^T
    wl1 = np.ascontiguousarray(ws["W1_1"].astype(np.float32).T)

    nc = _build_program()

    in_maps = []
    for cix in range(NCORES):
        sl = slice(cix * BC, (cix + 1) * BC)
        xb0 = np.concatenate(
            [z[sl].T, cond[sl].T, np.ones((1, BC), np.float32)], axis=0)
        im = {"xb0": np.ascontiguousarray(xb0), "wl1": wl1,
              "onesrow": np.ones((1, BC), np.float32)}
        im.update(consts)
        in_maps.append(im)

    res = run_bass_kernel_spmd(nc, in_maps, core_ids=list(range(NCORES)),
                               **spmd_kwargs)
    out = np.concatenate(
        [res.results[cix]["out"][0] for cix in range(NCORES)])
    return (out + const_add).astype(np.float32), res


def kernel(**inputs):
    out, _ = _run(inputs)
    return out


if __name__ == "__main__":
    rng = np.random.default_rng(0)
    fake = {}
    sizes = [(WID, L + C), (WID, WID), (L, WID)]
    for idx, (o, inp) in enumerate(sizes):
        fake[f"W1_{idx}"] = rng.standard_normal((o, inp)).astype(np.float32) * 0.1
        fake[f"b1_{idx}"] = rng.standard_normal(o).astype(np.float32) * 0.1
        fake[f"W2_{idx}"] = rng.standard_normal((o, 1)).astype(np.float32) * 0.1
        fake[f"b2_{idx}"] = rng.standard_normal(o).astype(np.float32) * 0.1
        fake[f"W3_{idx}"] = rng.standard_normal((o, 1)).astype(np.float32) * 0.1
    fake["z"] = rng.standard_normal((B, L)).astype(np.float32)
    fake["cond"] = rng.standard_normal((B, C)).astype(np.float32)
    print(kernel(**fake)[:8])
